# revision 40
# baseline (speedup 1.0000x reference)
"""Multi-head attention forward on 8 Trainium2 NeuronCores (Bass/Tile).

Problem: B=2, S=2048, d_model=1024, 16 heads (depth 64), fp32.
  q/k/v = query @ W{q,k,v}; logits = q k^T / 8 + mask * -1e9;
  out = softmax(logits) v @ Wo.

Sharding (Megatron-style, hardcoded): core c handles batch b = c//4 and head
group hg = c%4 (4 heads = 256 of the 1024 head dims). Wq/Wk/Wv are
column-sharded, Wo row-sharded; each core emits a partial [S, 1024] output
(bf16) and the host sums the 4 partials per batch (the "all-reduce").

Per-core design (engine-balanced, all-bf16 datapath):
  * All PE operands are bf16 (lower power -> less HAM clock throttling,
    small LDWEIGHTS). Attention math runs transposed: qT/kT are [dh, S] so
    QK^T lands as logitsT [k, q] tiles straight off the PE.
  * The inner loop is software-pipelined: QK^T for unit i+1 issues before the
    exp/mask/AV chain of unit i, so the PE stays busy while ScalarE runs exp
    (the pace-setter at ~1.1us per [128,1024] tile).
  * exp splits between ScalarE (EXP activation, 14/16 tiles) and VectorE
    (Schraudolph bit-trick: u16 = logit*A + B reinterpreted as bf16, 2/16).
  * Softmax denominators ride a ones-column in V; they go partition-major via
    a DRAM bounce + XBAR transpose DMA, one [128,16] reciprocal per head
    pair, back to a row with one PE transpose + DMA, then a rank-1 matmul
    broadcast scales the attn rows in place.
  * DMA order keeps the mask (8MB) from starving the x/W loads; the mask
    buffer holds one q-chunk and is refreshed mid-flight for chunk 1.
  * Output projection is decomposed into single-matmul tasks interleaved into
    the attention stream (chunk 0's during chunk 1, chunk 1's g0-half late in
    chunk 1 with an SBUF accumulator; only the g1-half + adds run in the
    tail). Output is bf16; the host does the final f32 partial-sum.
"""

import sys

import numpy as np

sys.path.insert(0, "/opt/trn_rl_repo")

B = 2
S = 2048
D = 1024
HEADS = 16
DEPTH = 64
CORES = 8
HG = 4          # head groups (cores per batch)
HPC = 4         # heads per core
DH = HPC * DEPTH  # per-core head width = 256

# Schraudolph exp in bf16 bits: u16 = round(logit * SCHR_A + SCHR_B)
# exp(0.125*l) = 2^(0.125*l*log2 e); bf16 bits = 128*(bexp+mant/128)
SCHR_A = 0.125 * 128.0 / float(np.log(2.0))
SCHR_B = 127.0 * 128.0 - 5.7 + 0.5

# kb tiles handled by the DVE bit-trick exp instead of ScalarE (per 16)
DVE_KBS = (5, 11)

_CACHE = {}


def _build_program():
    import concourse.bass as bass  # noqa: F401  (registers engines)
    import concourse.mybir as mybir
    import concourse.tile as tile
    from concourse import bacc
    from concourse.bass_interp import get_hw_module
    from concourse.masks import make_identity

    dt = mybir.dt
    f32, bf16, u16 = dt.float32, dt.bfloat16, dt.uint16
    MULT = mybir.AluOpType.mult
    ADD = mybir.AluOpType.add
    EXP = mybir.ActivationFunctionType.Exp

    nc = bacc.Bacc(
        "TRN2",
        target_bir_lowering=False,
        debug=False,
        enable_asserts=True,
        num_devices=CORES,
    )

    f32r = dt.float32r
    xT = nc.dram_tensor("xT", [D, S], bf16, kind="ExternalInput").ap()
    imaskT = nc.dram_tensor("imaskT", [S, S], bf16, kind="ExternalInput").ap()
    wq = nc.dram_tensor("wq", [D, DH], bf16, kind="ExternalInput").ap()
    wk = nc.dram_tensor("wk", [D, DH], bf16, kind="ExternalInput").ap()
    wv = nc.dram_tensor("wv", [D, DH], bf16, kind="ExternalInput").ap()
    wo = nc.dram_tensor("wo", [DH, D], bf16, kind="ExternalInput").ap()
    vones = nc.dram_tensor("vones", [128, HPC, 1], bf16, kind="ExternalInput").ap()
    ones_rd = nc.dram_tensor("ones_rd", [1, DEPTH], bf16, kind="ExternalInput").ap()
    out = nc.dram_tensor("out", [S, D], bf16, kind="ExternalOutput").ap()

    with tile.TileContext(nc) as tc:
        with tc.tile_pool(name="persist", bufs=1) as pp:
            qT = [pp.tile([128, S], bf16, tag=f"qT{g}", name=f"qT{g}") for g in range(2)]
            kT = [pp.tile([128, S], bf16, tag=f"kT{g}", name=f"kT{g}") for g in range(2)]
            vta = pp.tile([128, 16, HPC, DEPTH + 1], bf16, tag="vta", name="vta")
            wot = [pp.tile([128, D], bf16, tag=f"wo{g}", name=f"wo{g}") for g in range(2)]
            # per (qcp, g): attn rows for heads 2g, 2g+1 (normalized in place)
            ath = [[pp.tile([128, 1024], bf16, tag=f"ath{qc}{g}", name=f"ath{qc}{g}")
                    for g in range(2)] for qc in range(2)]
            mt = pp.tile([128, 16, 1024], bf16, tag="mask", name="mask")
            identb = pp.tile([128, 128], bf16, tag="identb", name="identb")
            dummy = pp.tile([1, 64], bf16, tag="dummy", name="dummy")
            ones_r = pp.tile([1, DEPTH], bf16, tag="ones_r", name="ones_r")
            one1b = pp.tile([1, 1], bf16, tag="one1b", name="one1b")
            # per-qcp denominator staging: row-major [1,1024] per head,
            # partition-major [128, 4h x 8c] for the reciprocal, and the
            # broadcast row [1, 4096] feeding the rank-1 psc matmuls.
            dden = [pp.tile([1, HPC, 1024], bf16, tag=f"dd{qc}", name=f"dd{qc}")
                    for qc in range(2)]
            den_pm = [pp.tile([128, 32], bf16, tag=f"dpm{qc}", name=f"dpm{qc}")
                      for qc in range(2)]
            rden_pm = [pp.tile([128, 32], bf16, tag=f"rpm{qc}", name=f"rpm{qc}")
                       for qc in range(2)]
            rrow = [pp.tile([1, 4096], bf16, tag=f"rr{qc}", name=f"rr{qc}")
                    for qc in range(2)]
            rt16 = pp.tile([16, 128], bf16, tag="rt16", name="rt16")
            dsc_cm = tc.tile_pool(name="dscr", bufs=1, space="DRAM")
            dsc = dsc_cm.__enter__()
            den_scr = [dsc.tile([32, 128], bf16, tag=f"ds{qc}", name=f"ds{qc}")
                       for qc in range(2)]

            # ---- DMA issue order: tiny, wk, xT, wq, wv, mask, wo ----
            nc.sync.dma_start(ones_r[:], ones_rd[:])
            ab_cm = tc.tile_pool(name="attn", bufs=1)
            ab = ab_cm.__enter__()
            exs_cm = tc.tile_pool(name="exs", bufs=3)
            exs = exs_cm.__enter__()
            wts = {}
            xtp = tc.tile_pool(name="xw", bufs=1)
            xw = xtp.__enter__()
            xt = [xw.tile([128, S], bf16, tag=f"x{d}", name=f"x{d}") for d in range(8)]
            for nm in ("wq", "wk", "wv"):
                wts[nm] = [xw.tile([128, DH], bf16, tag=f"{nm}{d}", name=f"{nm}{d}") for d in range(8)]
            for d in range(8):
                nc.sync.dma_start(wts["wk"][d][:], wk[d * 128:(d + 1) * 128, :])
            for d in range(8):
                nc.sync.dma_start(xt[d][:], xT[d * 128:(d + 1) * 128, :])
            for d in range(8):
                nc.sync.dma_start(wts["wv"][d][:], wv[d * 128:(d + 1) * 128, :])
            for d in range(8):
                nc.sync.dma_start(wts["wq"][d][:], wq[d * 128:(d + 1) * 128, :])
            imaskT_r = imaskT.rearrange("(t p) q -> p t q", p=128)
            for kb in range(16):
                nc.sync.dma_start(mt[:, kb:kb + 1, :],
                                  imaskT_r[:, kb:kb + 1, 0:1024])
            for g in range(2):
                nc.sync.dma_start(wot[g][:], wo[g * 128:(g + 1) * 128, :])
            for st in range(16):
                nc.sync.dma_start(vta[:, st, :, DEPTH:DEPTH + 1], vones[:])

            # identity (bf16) + exp-table preload + PE warm spin
            make_identity(nc, identb[:])
            nc.gpsimd.memset(one1b[:], 1.0)
            nc.scalar.activation(dummy[:], identb[0:1, 0:64], EXP, scale=1.0)
            with tc.tile_pool(name="psW", bufs=2, space="PSUM") as psW:
                for w in range(16):
                    psw = psW.tile([128, 128], f32, tag="warm", name="warm")
                    nc.tensor.matmul(psw[:], identb[:], identb[:],
                                     start=True, stop=True)

            # ---- projections (k, then q chunks sc0/sc1, then v) ----
            psA_cm = tc.tile_pool(name="psA", bufs=6, space="PSUM")
            psA = psA_cm.__enter__()

            def proj_qk(wt, dst, g, sc, pool, tag="proj"):
                ps = pool.tile([128, 512], f32, tag=tag, name=tag)
                for d in range(8):
                    nc.tensor.matmul(
                        ps[:],
                        wt[d][:, g * 128:(g + 1) * 128],
                        xt[d][:, sc * 512:(sc + 1) * 512],
                        start=(d == 0), stop=(d == 7),
                    )
                nc.vector.tensor_copy(dst[g][:, sc * 512:(sc + 1) * 512], ps[:])

            for g in range(2):
                for sc in range(4):
                    proj_qk(wts["wk"], kT, g, sc, psA)
            for sp in range(8):
                ps = psA.tile([128, 512], f32, tag="proj", name="proj")
                for sub in range(2):
                    st = 2 * sp + sub
                    for d in range(8):
                        nc.tensor.matmul(
                            ps[:, sub * 256:(sub + 1) * 256],
                            xt[d][:, st * 128:(st + 1) * 128],
                            wts["wv"][d][:],
                            start=(d == 0), stop=(d == 7),
                        )
                dst = vta[:, 2 * sp:2 * sp + 2, :, 0:DEPTH]
                srcv = ps[:].rearrange("p (s h e) -> p s h e", s=2, h=HPC)
                if sp % 2 == 0:
                    nc.vector.tensor_copy(dst, srcv)
                else:
                    nc.scalar.copy(dst, srcv)
            for g in range(2):
                for sc in range(2):
                    proj_qk(wts["wq"], qT, g, sc, psA)
            psA_cm.__exit__(None, None, None)

            # ---- attention ----
            with tc.tile_pool(name="psB", bufs=1, space="PSUM") as psB:
              psLO_cm = [tc.tile_pool(name="psL", bufs=2, space="PSUM"),
                         tc.tile_pool(name="psO", bufs=1, space="PSUM")]
              psL = psLO_cm[0].__enter__()
              psO = psLO_cm[1].__enter__()

              def emit_qk(qcp, h, kb):
                  g, po = h // 2, (h % 2) * 64
                  psl = psL.tile([128, 1024], f32, tag="lg", name="lg")
                  for half in range(2):
                      hs = slice(half * 512, (half + 1) * 512)
                      qh = slice(qcp * 1024 + half * 512,
                                 qcp * 1024 + half * 512 + 512)
                      nc.tensor.matmul(
                          psl[:, hs],
                          kT[g][po:po + 64, kb * 128:(kb + 1) * 128],
                          qT[g][po:po + 64, qh],
                          start=True, stop=True,
                      )
                  return psl

              # deferred PE side-tasks, popped one per inner unit
              pe_tasks = []

              q23_ps = {}

              def q23_mm(g, sc, dlo):
                  if dlo == 0:
                      q23_ps[(g, sc)] = psB.tile([128, 512], f32, tag="po",
                                                 name="po")
                  ps = q23_ps[(g, sc)]
                  for d in range(dlo, dlo + 4):
                      nc.tensor.matmul(
                          ps[:],
                          wts["wq"][d][:, g * 128:(g + 1) * 128],
                          xt[d][:, sc * 512:(sc + 1) * 512],
                          start=(d == 0), stop=(d == 7),
                      )

              def q23_copy(g, sc):
                  nc.vector.tensor_copy(
                      qT[g][:, sc * 512:(sc + 1) * 512], q23_ps.pop((g, sc)))

              def emit_q23():
                  for g in range(2):
                      for sc in range(2, 4):
                          pe_tasks.append(lambda g=g, sc=sc: q23_mm(g, sc, 0))
                          pe_tasks.append(lambda g=g, sc=sc: q23_mm(g, sc, 4))
                          pe_tasks.append(lambda g=g, sc=sc: q23_copy(g, sc))

              def outproj_nch(qcp, st, nch, ot, pool, tag):
                  psf = pool.tile([128, 512], f32, tag=tag, name="po")
                  for g in range(2):
                      nc.tensor.matmul(
                          psf[:],
                          ath[qcp][g][:, st * 128 - qcp * 1024:
                                      st * 128 - qcp * 1024 + 128],
                          wot[g][:, nch * 512:(nch + 1) * 512],
                          start=(g == 0), stop=(g == 1),
                      )
                  if nch == 0:
                      nc.scalar.copy(ot[:, 0:512], psf[:])
                  else:
                      nc.vector.tensor_copy(ot[:, 512:1024], psf[:])
                      nc.sync.dma_start(out[st * 128:(st + 1) * 128, :], ot[:])

              def emit_outproj(qcp, pool, tag):
                  for st in range(qcp * 8, qcp * 8 + 8):
                      def mk(st, nch):
                          def f(st=st, nch=nch):
                              mk_otq()
                              outproj_nch(qcp, st, nch, otq1[st % 8], pool,
                                          tag)
                          return f
                      pe_tasks.append(mk(st, 0))
                      pe_tasks.append(mk(st, 1))

              otq1 = []

              def mk_otq():
                  if not otq1:
                      for s in range(8):
                          otq1.append(ab.tile([128, D], bf16, tag=f"oq{s}",
                                              name=f"oq{s}", bufs=1))

              def og0_task(st, nch):
                  mk_otq()
                  psf = psB.tile([128, 512], f32, tag="po", name="po")
                  lo = st * 128 - 1024
                  nc.tensor.matmul(
                      psf[:], ath[1][0][:, lo:lo + 128],
                      wot[0][:, nch * 512:(nch + 1) * 512],
                      start=True, stop=True)
                  if nch == 0:
                      nc.scalar.copy(otq1[st - 8][:, 0:512], psf[:])
                  else:
                      nc.vector.tensor_copy(otq1[st - 8][:, 512:1024], psf[:])

              def og1_tail(pool, tag):
                  for st in range(8, 16):
                      lo = st * 128 - 1024
                      for nch in range(2):
                          hs = slice(nch * 512, (nch + 1) * 512)
                          psf = pool.tile([128, 512], f32, tag=tag, name="po")
                          nc.tensor.matmul(
                              psf[:], ath[1][1][:, lo:lo + 128],
                              wot[1][:, nch * 512:(nch + 1) * 512],
                              start=True, stop=True)
                          nc.vector.tensor_tensor(
                              otq1[st - 8][:, hs], otq1[st - 8][:, hs],
                              psf[:], ADD)
                      nc.sync.dma_start(out[st * 128:(st + 1) * 128, :],
                                        otq1[st - 8][:])

              def head_den_dma(qcp, h):
                  # park this head's denominator row in DRAM scratch
                  nc.sync.dma_start(den_scr[qcp][h * 8:(h + 1) * 8, :],
                                    dden[qcp][0:1, h, :])

              def pair_gather(qcp, g):
                  # XBAR-transpose both heads' dens to partition-major and
                  # take one [128,16] reciprocal.
                  sl = slice(g * 16, (g + 1) * 16)
                  nc.sync.dma_start_transpose(den_pm[qcp][:, sl],
                                              den_scr[qcp][sl, :])
                  with nc.allow_low_precision(reason="bf16 denominators"):
                      nc.vector.reciprocal(rden_pm[qcp][:, sl],
                                           den_pm[qcp][:, sl])

              def pair_rows(qcp, g, pool, tag):
                  # one [128,16] PE transpose, then a psum->sbuf DMA
                  # linearizes both heads' reciprocals into the rrow.
                  sl = slice(g * 16, (g + 1) * 16)
                  ptr = pool.tile([128, 512], bf16, tag=f"{tag}dt", name="ptr")
                  nc.tensor.transpose(ptr[0:16, 0:128], rden_pm[qcp][:, sl],
                                      identb[:])
                  nc.scalar.copy(rt16[:], ptr[0:16, 0:128])
                  nc.sync.dma_start(
                      rrow[qcp][0:1, 2 * g * 1024:(2 * g + 2) * 1024],
                      rt16[:])

              def head_norm_scale(qcp, h, pool, tag):
                  # rank-1 broadcast into psum, scale attn rows in place
                  g, po = h // 2, (h % 2) * 64
                  for half in range(2):
                      hs = slice(half * 512, (half + 1) * 512)
                      psc = pool.tile([128, 512], f32, tag=tag, name="psc")
                      nc.tensor.matmul(
                          psc[po:po + 64, :],
                          ones_r[:],
                          rrow[qcp][0:1, h * 1024 + half * 512:
                                    h * 1024 + half * 512 + 512],
                          start=True, stop=True,
                      )
                      nc.vector.tensor_tensor(
                          ath[qcp][g][po:po + 64, hs],
                          ath[qcp][g][po:po + 64, hs], psc[po:po + 64, :],
                          MULT)

              def head_fast_epi(qcp, h, pool, tag):
                  # forward transposes on the PE (no DRAM hop): dden row ->
                  # partition-major psum, reciprocal straight from psum.
                  # bf16 columns land on 4-byte boundaries via an f32 bitcast.
                  pst = pool.tile([128, 8], f32, tag=tag, name="fwd")
                  v = pst[:].bitcast(bf16)
                  for c in range(8):
                      nc.tensor.transpose(
                          v[:, 2 * c:2 * c + 1],
                          dden[qcp][0:1, h, c * 128:(c + 1) * 128],
                          one1b[:],
                      )
                  vin = v.rearrange("p (c two) -> p c two", two=2)[:, :, 0:1]
                  with nc.allow_low_precision(reason="bf16 denominators"):
                      nc.vector.reciprocal(
                          rden_pm[qcp][:, h * 8:(h + 1) * 8].unsqueeze(2), vin)

              pend = []  # (due_global_unit, closure)
              gu = 0
              for qcp in range(2):
                  if qcp == 0:
                      emit_q23()
                  else:
                      xtp.__exit__(None, None, None)
                      emit_outproj(0, psB, "po")
                      for st in range(8, 16):
                          for nch in range(2):
                              pe_tasks.append(
                                  lambda st=st, nch=nch: og0_task(st, nch))
                  units = [(h, kb) for h in range(HPC) for kb in range(16)]
                  psl_next = emit_qk(qcp, 0, 0)
                  for i, (h, kb) in enumerate(units):
                      g, po = h // 2, (h % 2) * 64
                      psl = psl_next
                      if i + 1 < len(units):
                          hn, kbn = units[i + 1]
                          psl_next = emit_qk(qcp, hn, kbn)
                      ex = exs.tile([128, 1024], bf16, tag="ex", name="ex", bufs=4)
                      if (kb % 16) in DVE_KBS:
                          nc.vector.tensor_scalar(
                              ex[:].bitcast(u16), psl[:],
                              SCHR_A, SCHR_B, MULT, ADD)
                      else:
                          nc.scalar.activation(ex[:], psl[:], EXP, scale=0.125)
                      em = exs.tile([128, 1024], bf16, tag="em", name="em", bufs=4)
                      nc.vector.tensor_tensor(em[:], ex[:], mt[:, kb, :], MULT)
                      if qcp == 0 and h == 3:
                          # refresh this kb's mask tile with the q-chunk-1
                          # columns now that chunk 0 is done with it
                          nc.sync.dma_start(
                              mt[:, kb:kb + 1, :],
                              imaskT_r[:, kb:kb + 1, 1024:2048])
                      for half in range(2):
                          hs = slice(half * 512, (half + 1) * 512)
                          pso = psO.tile([65, 512], f32, tag=f"av{half}",
                                         name=f"av{half}")
                          nc.tensor.matmul(
                              pso[:], vta[:, kb, h, :], em[:, hs],
                              start=(kb == 0), stop=(kb == 15),
                          )
                          if kb == 15:
                              nc.vector.tensor_copy(
                                  ath[qcp][g][po:po + 64, hs], pso[0:64, :])
                              nc.scalar.copy(
                                  dden[qcp][0:1, h, hs], pso[64:65, :])
                      if kb == 15:
                          if qcp == 1 and h >= 2:
                              if h == 2:
                                  pend.append((gu + 2, lambda:
                                               head_fast_epi(1, 2, psB, "po")))
                          else:
                              head_den_dma(qcp, h)
                              if h % 2 == 1:
                                  pair_gather(qcp, g)
                                  pend.append((gu + 6, lambda qcp=qcp, g=g:
                                               pair_rows(qcp, g, psB, "po")))
                                  pend.append((gu + 8, lambda qcp=qcp, g=g: (
                                      head_norm_scale(qcp, 2 * g, psB, "po"),
                                      head_norm_scale(qcp, 2 * g + 1, psB, "po"))))
                      if pend and pend[0][0] <= gu:
                          pend.pop(0)[1]()
                      if pe_tasks and i % 2 == 1 and (qcp == 0 or i >= 9):
                          pe_tasks.pop(0)()
                      gu += 1
                  while qcp == 0 and pe_tasks:
                      pe_tasks.pop(0)()
              while pend:
                  pend.pop(0)[1]()
              while pe_tasks:
                  pe_tasks.pop(0)()
              psLO_cm[1].__exit__(None, None, None)
              psLO_cm[0].__exit__(None, None, None)
              with tc.tile_pool(name="psT", bufs=3, space="PSUM") as psT:
                  head_fast_epi(1, 3, psT, "pt")
                  pair_rows(1, 1, psT, "pt")
                  head_norm_scale(1, 2, psT, "pt")
                  head_norm_scale(1, 3, psT, "pt")
                  og1_tail(psT, "pt")
                  while pe_tasks:
                      pe_tasks.pop(0)()
            exs_cm.__exit__(None, None, None)
            ab_cm.__exit__(None, None, None)
            dsc_cm.__exit__(None, None, None)

    nc.compile()
    nc.m = get_hw_module(nc.m)
    return nc


def _get_program():
    if "nc" not in _CACHE:
        _CACHE["nc"] = _build_program()
    return _CACHE["nc"]


def _make_in_maps(query, attention_mask, Wq, Wk, Wv, Wo):
    import ml_dtypes

    bf = ml_dtypes.bfloat16
    in_maps = []
    imaskT_b = []
    xT_b = []
    for b in range(B):
        imaskT_b.append(
            np.ascontiguousarray(1 - attention_mask[b, 0].T).astype(bf)
        )
        xT_b.append(np.ascontiguousarray(query[b].T).astype(bf))
    for c in range(CORES):
        b, hg = c // HG, c % HG
        cs = slice(hg * DH, (hg + 1) * DH)
        in_maps.append({
            "xT": xT_b[b],
            "imaskT": imaskT_b[b],
            "wq": np.ascontiguousarray(Wq[:, cs]).astype(bf),
            "wk": np.ascontiguousarray(Wk[:, cs]).astype(bf),
            "wv": np.ascontiguousarray(Wv[:, cs]).astype(bf),
            "wo": np.ascontiguousarray(Wo[cs, :]).astype(bf),
            "vones": np.ones((128, HPC, 1), dtype=bf),
            "ones_rd": np.ones((1, DEPTH), dtype=bf),
        })
    return in_maps


def _run(inputs, trace=False):
    from concourse.bass_utils import run_bass_kernel_spmd

    nc = _get_program()
    in_maps = _make_in_maps(**inputs)
    res = run_bass_kernel_spmd(
        nc, in_maps, core_ids=list(range(CORES)), trace=trace,
    )
    outs = [res.results[c]["out"].astype(np.float32) for c in range(CORES)]
    full = np.empty((B, S, D), dtype=np.float32)
    for b in range(B):
        acc = outs[4 * b]
        for hg in range(1, HG):
            acc = acc + outs[4 * b + hg]
        full[b] = acc
    return full, res


def kernel(query, attention_mask, Wq, Wk, Wv, Wo):
    full, _ = _run(dict(
        query=np.asarray(query), attention_mask=np.asarray(attention_mask),
        Wq=np.asarray(Wq), Wk=np.asarray(Wk), Wv=np.asarray(Wv),
        Wo=np.asarray(Wo),
    ))
    return full


# revision 41
# speedup vs baseline: 1.0211x; 1.0211x over previous
"""Multi-head attention forward on 8 Trainium2 NeuronCores (Bass/Tile).

Problem: B=2, S=2048, d_model=1024, 16 heads (depth 64), fp32.
  q/k/v = query @ W{q,k,v}; logits = q k^T / 8 + mask * -1e9;
  out = softmax(logits) v @ Wo.

Sharding (Megatron-style, hardcoded): core c handles batch b = c//4 and head
group hg = c%4 (4 heads = 256 of the 1024 head dims). Wq/Wk/Wv are
column-sharded, Wo row-sharded; each core emits a partial [S, 1024] output
(bf16) and the host sums the 4 partials per batch (the "all-reduce").

Per-core design (engine-balanced, all-bf16 datapath):
  * All PE operands are bf16 (lower power -> less HAM clock throttling,
    small LDWEIGHTS). Attention math runs transposed: qT/kT are [dh, S] so
    QK^T lands as logitsT [k, q] tiles straight off the PE.
  * The inner loop is software-pipelined: QK^T for unit i+1 issues before the
    exp/mask/AV chain of unit i, so the PE stays busy while ScalarE runs exp
    (the pace-setter at ~1.1us per [128,1024] tile).
  * exp splits between ScalarE (EXP activation, 14/16 tiles) and VectorE
    (Schraudolph bit-trick: u16 = logit*A + B reinterpreted as bf16, 2/16).
  * Softmax denominators ride a ones-column in V; they go partition-major via
    a DRAM bounce + XBAR transpose DMA, one [128,16] reciprocal per head
    pair, back to a row with one PE transpose + DMA, then a rank-1 matmul
    broadcast scales the attn rows in place.
  * DMA order keeps the mask (8MB) from starving the x/W loads; the mask
    buffer holds one q-chunk and is refreshed mid-flight for chunk 1.
  * Output projection is decomposed into single-matmul tasks interleaved into
    the attention stream (chunk 0's during chunk 1, chunk 1's g0-half late in
    chunk 1 with an SBUF accumulator; only the g1-half + adds run in the
    tail). Output is bf16; the host does the final f32 partial-sum.
"""

import sys

import numpy as np

sys.path.insert(0, "/opt/trn_rl_repo")

B = 2
S = 2048
D = 1024
HEADS = 16
DEPTH = 64
CORES = 8
HG = 4          # head groups (cores per batch)
HPC = 4         # heads per core
DH = HPC * DEPTH  # per-core head width = 256

# Schraudolph exp in bf16 bits: u16 = round(logit * SCHR_A + SCHR_B)
# exp(0.125*l) = 2^(0.125*l*log2 e); bf16 bits = 128*(bexp+mant/128)
SCHR_A = 0.125 * 128.0 / float(np.log(2.0))
SCHR_B = 127.0 * 128.0 - 5.7 + 0.5

# kb tiles handled by the DVE bit-trick exp instead of ScalarE (per 16)
DVE_KBS = (5, 11)

_CACHE = {}


def _build_program():
    import concourse.bass as bass  # noqa: F401  (registers engines)
    import concourse.mybir as mybir
    import concourse.tile as tile
    from concourse import bacc
    from concourse.bass_interp import get_hw_module
    from concourse.masks import make_identity

    dt = mybir.dt
    f32, bf16, u16 = dt.float32, dt.bfloat16, dt.uint16
    MULT = mybir.AluOpType.mult
    ADD = mybir.AluOpType.add
    EXP = mybir.ActivationFunctionType.Exp

    nc = bacc.Bacc(
        "TRN2",
        target_bir_lowering=False,
        debug=False,
        enable_asserts=True,
        num_devices=CORES,
    )

    f32r = dt.float32r
    xT = nc.dram_tensor("xT", [D, S], bf16, kind="ExternalInput").ap()
    imaskT = nc.dram_tensor("imaskT", [S, S], bf16, kind="ExternalInput").ap()
    wq = nc.dram_tensor("wq", [D, DH], bf16, kind="ExternalInput").ap()
    wk = nc.dram_tensor("wk", [D, DH], bf16, kind="ExternalInput").ap()
    wv = nc.dram_tensor("wv", [D, DH], bf16, kind="ExternalInput").ap()
    wo = nc.dram_tensor("wo", [DH, D], bf16, kind="ExternalInput").ap()
    vones = nc.dram_tensor("vones", [128, HPC, 1], bf16, kind="ExternalInput").ap()
    ones_rd = nc.dram_tensor("ones_rd", [1, DEPTH], bf16, kind="ExternalInput").ap()
    out = nc.dram_tensor("out", [S, D], bf16, kind="ExternalOutput").ap()

    with tile.TileContext(nc) as tc:
        with tc.tile_pool(name="persist", bufs=1) as pp:
            qT = [pp.tile([128, S], bf16, tag=f"qT{g}", name=f"qT{g}") for g in range(2)]
            kT = [pp.tile([128, S], bf16, tag=f"kT{g}", name=f"kT{g}") for g in range(2)]
            vta = pp.tile([128, 16, HPC, DEPTH + 1], bf16, tag="vta", name="vta")
            wot = [pp.tile([128, D], bf16, tag=f"wo{g}", name=f"wo{g}") for g in range(2)]
            # per (qcp, g): attn rows for heads 2g, 2g+1 (normalized in place)
            ath = [[pp.tile([128, 1024], bf16, tag=f"ath{qc}{g}", name=f"ath{qc}{g}")
                    for g in range(2)] for qc in range(2)]
            mt = pp.tile([128, 16, 1024], bf16, tag="mask", name="mask")
            identb = pp.tile([128, 128], bf16, tag="identb", name="identb")
            dummy = pp.tile([1, 64], bf16, tag="dummy", name="dummy")
            ones_r = pp.tile([1, DEPTH], bf16, tag="ones_r", name="ones_r")
            one1b = pp.tile([1, 1], bf16, tag="one1b", name="one1b")
            # per-qcp denominator staging: row-major [1,1024] per head,
            # partition-major [128, 4h x 8c] for the reciprocal, and the
            # broadcast row [1, 4096] feeding the rank-1 psc matmuls.
            dden = [pp.tile([1, HPC, 1024], bf16, tag=f"dd{qc}", name=f"dd{qc}")
                    for qc in range(2)]
            den_pm = [pp.tile([128, 32], bf16, tag=f"dpm{qc}", name=f"dpm{qc}")
                      for qc in range(2)]
            rden_pm = [pp.tile([128, 32], bf16, tag=f"rpm{qc}", name=f"rpm{qc}")
                       for qc in range(2)]
            rrow = [pp.tile([1, 4096], bf16, tag=f"rr{qc}", name=f"rr{qc}")
                    for qc in range(2)]
            rt16 = pp.tile([16, 128], bf16, tag="rt16", name="rt16")
            dsc_cm = tc.tile_pool(name="dscr", bufs=1, space="DRAM")
            dsc = dsc_cm.__enter__()
            den_scr = [dsc.tile([32, 128], bf16, tag=f"ds{qc}", name=f"ds{qc}")
                       for qc in range(2)]

            # ---- DMA issue order: tiny, wk, xT, wq, wv, mask, wo ----
            nc.sync.dma_start(ones_r[:], ones_rd[:])
            ab_cm = tc.tile_pool(name="attn", bufs=1)
            ab = ab_cm.__enter__()
            exs_cm = tc.tile_pool(name="exs", bufs=3)
            exs = exs_cm.__enter__()
            wts = {}
            xtp = tc.tile_pool(name="xw", bufs=1)
            xw = xtp.__enter__()
            xt = [xw.tile([128, S], bf16, tag=f"x{d}", name=f"x{d}") for d in range(8)]
            for nm in ("wq", "wk", "wv"):
                wts[nm] = [xw.tile([128, DH], bf16, tag=f"{nm}{d}", name=f"{nm}{d}") for d in range(8)]
            for d in range(8):
                nc.sync.dma_start(wts["wk"][d][:], wk[d * 128:(d + 1) * 128, :])
            for d in range(8):
                nc.sync.dma_start(xt[d][:], xT[d * 128:(d + 1) * 128, :])
            for d in range(8):
                nc.sync.dma_start(wts["wv"][d][:], wv[d * 128:(d + 1) * 128, :])
            for d in range(8):
                nc.sync.dma_start(wts["wq"][d][:], wq[d * 128:(d + 1) * 128, :])
            imaskT_r = imaskT.rearrange("(t p) q -> p t q", p=128)
            for kb in range(16):
                nc.sync.dma_start(mt[:, kb:kb + 1, :],
                                  imaskT_r[:, kb:kb + 1, 0:1024])
            for g in range(2):
                nc.sync.dma_start(wot[g][:], wo[g * 128:(g + 1) * 128, :])
            for st in range(16):
                nc.sync.dma_start(vta[:, st, :, DEPTH:DEPTH + 1], vones[:])

            # identity (bf16) + exp-table preload + PE warm spin
            make_identity(nc, identb[:])
            nc.gpsimd.memset(one1b[:], 1.0)
            nc.scalar.activation(dummy[:], identb[0:1, 0:64], EXP, scale=1.0)
            with tc.tile_pool(name="psW", bufs=2, space="PSUM") as psW:
                for w in range(40):
                    psw = psW.tile([128, 128], f32, tag="warm", name="warm")
                    nc.tensor.matmul(psw[:], identb[:], identb[:],
                                     start=True, stop=True)

            # ---- projections (k, then q chunks sc0/sc1, then v) ----
            psA_cm = tc.tile_pool(name="psA", bufs=6, space="PSUM")
            psA = psA_cm.__enter__()

            def proj_qk(wt, dst, g, sc, pool, tag="proj"):
                ps = pool.tile([128, 512], f32, tag=tag, name=tag)
                for d in range(8):
                    nc.tensor.matmul(
                        ps[:],
                        wt[d][:, g * 128:(g + 1) * 128],
                        xt[d][:, sc * 512:(sc + 1) * 512],
                        start=(d == 0), stop=(d == 7),
                    )
                nc.vector.tensor_copy(dst[g][:, sc * 512:(sc + 1) * 512], ps[:])

            for g in range(2):
                for sc in range(4):
                    proj_qk(wts["wk"], kT, g, sc, psA)
            for sp in range(8):
                ps = psA.tile([128, 512], f32, tag="proj", name="proj")
                for sub in range(2):
                    st = 2 * sp + sub
                    for d in range(8):
                        nc.tensor.matmul(
                            ps[:, sub * 256:(sub + 1) * 256],
                            xt[d][:, st * 128:(st + 1) * 128],
                            wts["wv"][d][:],
                            start=(d == 0), stop=(d == 7),
                        )
                dst = vta[:, 2 * sp:2 * sp + 2, :, 0:DEPTH]
                srcv = ps[:].rearrange("p (s h e) -> p s h e", s=2, h=HPC)
                if sp % 2 == 0:
                    nc.vector.tensor_copy(dst, srcv)
                else:
                    nc.scalar.copy(dst, srcv)
            for g in range(2):
                for sc in range(2):
                    proj_qk(wts["wq"], qT, g, sc, psA)
            psA_cm.__exit__(None, None, None)

            # ---- attention ----
            with tc.tile_pool(name="psB", bufs=1, space="PSUM") as psB:
              psLO_cm = [tc.tile_pool(name="psL", bufs=2, space="PSUM"),
                         tc.tile_pool(name="psO", bufs=1, space="PSUM")]
              psL = psLO_cm[0].__enter__()
              psO = psLO_cm[1].__enter__()

              def emit_qk(qcp, h, kb):
                  g, po = h // 2, (h % 2) * 64
                  psl = psL.tile([128, 1024], f32, tag="lg", name="lg")
                  for half in range(2):
                      hs = slice(half * 512, (half + 1) * 512)
                      qh = slice(qcp * 1024 + half * 512,
                                 qcp * 1024 + half * 512 + 512)
                      nc.tensor.matmul(
                          psl[:, hs],
                          kT[g][po:po + 64, kb * 128:(kb + 1) * 128],
                          qT[g][po:po + 64, qh],
                          start=True, stop=True,
                      )
                  return psl

              # deferred PE side-tasks, popped one per inner unit
              pe_tasks = []

              q23_ps = {}

              def q23_mm(g, sc, dlo):
                  if dlo == 0:
                      q23_ps[(g, sc)] = psB.tile([128, 512], f32, tag="po",
                                                 name="po")
                  ps = q23_ps[(g, sc)]
                  for d in range(dlo, dlo + 4):
                      nc.tensor.matmul(
                          ps[:],
                          wts["wq"][d][:, g * 128:(g + 1) * 128],
                          xt[d][:, sc * 512:(sc + 1) * 512],
                          start=(d == 0), stop=(d == 7),
                      )

              def q23_copy(g, sc):
                  nc.vector.tensor_copy(
                      qT[g][:, sc * 512:(sc + 1) * 512], q23_ps.pop((g, sc)))

              def emit_q23():
                  for g in range(2):
                      for sc in range(2, 4):
                          pe_tasks.append(lambda g=g, sc=sc: q23_mm(g, sc, 0))
                          pe_tasks.append(lambda g=g, sc=sc: q23_mm(g, sc, 4))
                          pe_tasks.append(lambda g=g, sc=sc: q23_copy(g, sc))

              def outproj_nch(qcp, st, nch, ot, pool, tag):
                  psf = pool.tile([128, 512], f32, tag=tag, name="po")
                  for g in range(2):
                      nc.tensor.matmul(
                          psf[:],
                          ath[qcp][g][:, st * 128 - qcp * 1024:
                                      st * 128 - qcp * 1024 + 128],
                          wot[g][:, nch * 512:(nch + 1) * 512],
                          start=(g == 0), stop=(g == 1),
                      )
                  if nch == 0:
                      nc.scalar.copy(ot[:, 0:512], psf[:])
                  else:
                      nc.vector.tensor_copy(ot[:, 512:1024], psf[:])
                      nc.sync.dma_start(out[st * 128:(st + 1) * 128, :], ot[:])

              def emit_outproj(qcp, pool, tag):
                  for st in range(qcp * 8, qcp * 8 + 8):
                      def mk(st, nch):
                          def f(st=st, nch=nch):
                              mk_otq()
                              outproj_nch(qcp, st, nch, otq1[st % 8], pool,
                                          tag)
                          return f
                      pe_tasks.append(mk(st, 0))
                      pe_tasks.append(mk(st, 1))

              otq1 = []

              def mk_otq():
                  if not otq1:
                      for s in range(8):
                          otq1.append(ab.tile([128, D], bf16, tag=f"oq{s}",
                                              name=f"oq{s}", bufs=1))

              def og0_task(st, nch):
                  mk_otq()
                  psf = psB.tile([128, 512], f32, tag="po", name="po")
                  lo = st * 128 - 1024
                  nc.tensor.matmul(
                      psf[:], ath[1][0][:, lo:lo + 128],
                      wot[0][:, nch * 512:(nch + 1) * 512],
                      start=True, stop=True)
                  if nch == 0:
                      nc.scalar.copy(otq1[st - 8][:, 0:512], psf[:])
                  else:
                      nc.vector.tensor_copy(otq1[st - 8][:, 512:1024], psf[:])

              def og1_tail(pool, tag):
                  for st in range(8, 16):
                      lo = st * 128 - 1024
                      for nch in range(2):
                          hs = slice(nch * 512, (nch + 1) * 512)
                          psf = pool.tile([128, 512], f32, tag=tag, name="po")
                          nc.tensor.matmul(
                              psf[:], ath[1][1][:, lo:lo + 128],
                              wot[1][:, nch * 512:(nch + 1) * 512],
                              start=True, stop=True)
                          nc.vector.tensor_tensor(
                              otq1[st - 8][:, hs], otq1[st - 8][:, hs],
                              psf[:], ADD)
                      nc.sync.dma_start(out[st * 128:(st + 1) * 128, :],
                                        otq1[st - 8][:])

              def head_den_dma(qcp, h):
                  # park this head's denominator row in DRAM scratch
                  nc.sync.dma_start(den_scr[qcp][h * 8:(h + 1) * 8, :],
                                    dden[qcp][0:1, h, :])

              def pair_gather(qcp, g):
                  # XBAR-transpose both heads' dens to partition-major and
                  # take one [128,16] reciprocal.
                  sl = slice(g * 16, (g + 1) * 16)
                  nc.sync.dma_start_transpose(den_pm[qcp][:, sl],
                                              den_scr[qcp][sl, :])
                  with nc.allow_low_precision(reason="bf16 denominators"):
                      nc.vector.reciprocal(rden_pm[qcp][:, sl],
                                           den_pm[qcp][:, sl])

              def pair_rows(qcp, g, pool, tag):
                  # one [128,16] PE transpose, then a psum->sbuf DMA
                  # linearizes both heads' reciprocals into the rrow.
                  sl = slice(g * 16, (g + 1) * 16)
                  ptr = pool.tile([128, 512], bf16, tag=f"{tag}dt", name="ptr")
                  nc.tensor.transpose(ptr[0:16, 0:128], rden_pm[qcp][:, sl],
                                      identb[:])
                  nc.scalar.copy(rt16[:], ptr[0:16, 0:128])
                  nc.sync.dma_start(
                      rrow[qcp][0:1, 2 * g * 1024:(2 * g + 2) * 1024],
                      rt16[:])

              def head_norm_scale(qcp, h, pool, tag):
                  # rank-1 broadcast into psum, scale attn rows in place
                  g, po = h // 2, (h % 2) * 64
                  for half in range(2):
                      hs = slice(half * 512, (half + 1) * 512)
                      psc = pool.tile([128, 512], f32, tag=tag, name="psc")
                      nc.tensor.matmul(
                          psc[po:po + 64, :],
                          ones_r[:],
                          rrow[qcp][0:1, h * 1024 + half * 512:
                                    h * 1024 + half * 512 + 512],
                          start=True, stop=True,
                      )
                      nc.vector.tensor_tensor(
                          ath[qcp][g][po:po + 64, hs],
                          ath[qcp][g][po:po + 64, hs], psc[po:po + 64, :],
                          MULT)

              def head_fast_epi(qcp, h, pool, tag):
                  # forward transposes on the PE (no DRAM hop): dden row ->
                  # partition-major psum, reciprocal straight from psum.
                  # bf16 columns land on 4-byte boundaries via an f32 bitcast.
                  pst = pool.tile([128, 8], f32, tag=tag, name="fwd")
                  v = pst[:].bitcast(bf16)
                  for c in range(8):
                      nc.tensor.transpose(
                          v[:, 2 * c:2 * c + 1],
                          dden[qcp][0:1, h, c * 128:(c + 1) * 128],
                          one1b[:],
                      )
                  vin = v.rearrange("p (c two) -> p c two", two=2)[:, :, 0:1]
                  with nc.allow_low_precision(reason="bf16 denominators"):
                      nc.vector.reciprocal(
                          rden_pm[qcp][:, h * 8:(h + 1) * 8].unsqueeze(2), vin)

              pend = []  # (due_global_unit, closure)
              gu = 0
              for qcp in range(2):
                  if qcp == 0:
                      emit_q23()
                  else:
                      xtp.__exit__(None, None, None)
                      emit_outproj(0, psB, "po")
                      for st in range(8, 16):
                          for nch in range(2):
                              pe_tasks.append(
                                  lambda st=st, nch=nch: og0_task(st, nch))
                  units = [(h, kb) for h in range(HPC) for kb in range(16)]
                  psl_next = emit_qk(qcp, 0, 0)
                  for i, (h, kb) in enumerate(units):
                      g, po = h // 2, (h % 2) * 64
                      psl = psl_next
                      if i + 1 < len(units):
                          hn, kbn = units[i + 1]
                          psl_next = emit_qk(qcp, hn, kbn)
                      ex = exs.tile([128, 1024], bf16, tag="ex", name="ex", bufs=4)
                      if (kb % 16) in DVE_KBS:
                          nc.vector.tensor_scalar(
                              ex[:].bitcast(u16), psl[:],
                              SCHR_A, SCHR_B, MULT, ADD)
                      else:
                          nc.scalar.activation(ex[:], psl[:], EXP, scale=0.125)
                      em = exs.tile([128, 1024], bf16, tag="em", name="em", bufs=4)
                      nc.vector.tensor_tensor(em[:], ex[:], mt[:, kb, :], MULT)
                      if qcp == 0 and h == 3:
                          # refresh this kb's mask tile with the q-chunk-1
                          # columns now that chunk 0 is done with it
                          nc.sync.dma_start(
                              mt[:, kb:kb + 1, :],
                              imaskT_r[:, kb:kb + 1, 1024:2048])
                      for half in range(2):
                          hs = slice(half * 512, (half + 1) * 512)
                          pso = psO.tile([65, 512], f32, tag=f"av{half}",
                                         name=f"av{half}")
                          nc.tensor.matmul(
                              pso[:], vta[:, kb, h, :], em[:, hs],
                              start=(kb == 0), stop=(kb == 15),
                          )
                          if kb == 15:
                              nc.vector.tensor_copy(
                                  ath[qcp][g][po:po + 64, hs], pso[0:64, :])
                              nc.scalar.copy(
                                  dden[qcp][0:1, h, hs], pso[64:65, :])
                      if kb == 15:
                          if qcp == 1 and h >= 2:
                              if h == 2:
                                  pend.append((gu + 2, lambda:
                                               head_fast_epi(1, 2, psB, "po")))
                          else:
                              head_den_dma(qcp, h)
                              if h % 2 == 1:
                                  pair_gather(qcp, g)
                                  pend.append((gu + 6, lambda qcp=qcp, g=g:
                                               pair_rows(qcp, g, psB, "po")))
                                  pend.append((gu + 8, lambda qcp=qcp, g=g: (
                                      head_norm_scale(qcp, 2 * g, psB, "po"),
                                      head_norm_scale(qcp, 2 * g + 1, psB, "po"))))
                      if pend and pend[0][0] <= gu:
                          pend.pop(0)[1]()
                      if pe_tasks and i % 2 == 1 and (qcp == 0 or i >= 9):
                          pe_tasks.pop(0)()
                      gu += 1
                  while qcp == 0 and pe_tasks:
                      pe_tasks.pop(0)()
              while pend:
                  pend.pop(0)[1]()
              while pe_tasks:
                  pe_tasks.pop(0)()
              psLO_cm[1].__exit__(None, None, None)
              psLO_cm[0].__exit__(None, None, None)
              with tc.tile_pool(name="psT", bufs=3, space="PSUM") as psT:
                  head_fast_epi(1, 3, psT, "pt")
                  pair_rows(1, 1, psT, "pt")
                  head_norm_scale(1, 2, psT, "pt")
                  head_norm_scale(1, 3, psT, "pt")
                  og1_tail(psT, "pt")
                  while pe_tasks:
                      pe_tasks.pop(0)()
            exs_cm.__exit__(None, None, None)
            ab_cm.__exit__(None, None, None)
            dsc_cm.__exit__(None, None, None)

    nc.compile()
    nc.m = get_hw_module(nc.m)
    return nc


def _get_program():
    if "nc" not in _CACHE:
        _CACHE["nc"] = _build_program()
    return _CACHE["nc"]


def _make_in_maps(query, attention_mask, Wq, Wk, Wv, Wo):
    import ml_dtypes

    bf = ml_dtypes.bfloat16
    in_maps = []
    imaskT_b = []
    xT_b = []
    for b in range(B):
        imaskT_b.append(
            np.ascontiguousarray(1 - attention_mask[b, 0].T).astype(bf)
        )
        xT_b.append(np.ascontiguousarray(query[b].T).astype(bf))
    for c in range(CORES):
        b, hg = c // HG, c % HG
        cs = slice(hg * DH, (hg + 1) * DH)
        in_maps.append({
            "xT": xT_b[b],
            "imaskT": imaskT_b[b],
            "wq": np.ascontiguousarray(Wq[:, cs]).astype(bf),
            "wk": np.ascontiguousarray(Wk[:, cs]).astype(bf),
            "wv": np.ascontiguousarray(Wv[:, cs]).astype(bf),
            "wo": np.ascontiguousarray(Wo[cs, :]).astype(bf),
            "vones": np.ones((128, HPC, 1), dtype=bf),
            "ones_rd": np.ones((1, DEPTH), dtype=bf),
        })
    return in_maps


def _run(inputs, trace=False):
    from concourse.bass_utils import run_bass_kernel_spmd

    nc = _get_program()
    in_maps = _make_in_maps(**inputs)
    res = run_bass_kernel_spmd(
        nc, in_maps, core_ids=list(range(CORES)), trace=trace,
    )
    outs = [res.results[c]["out"].astype(np.float32) for c in range(CORES)]
    full = np.empty((B, S, D), dtype=np.float32)
    for b in range(B):
        acc = outs[4 * b]
        for hg in range(1, HG):
            acc = acc + outs[4 * b + hg]
        full[b] = acc
    return full, res


def kernel(query, attention_mask, Wq, Wk, Wv, Wo):
    full, _ = _run(dict(
        query=np.asarray(query), attention_mask=np.asarray(attention_mask),
        Wq=np.asarray(Wq), Wk=np.asarray(Wk), Wv=np.asarray(Wv),
        Wo=np.asarray(Wo),
    ))
    return full
